# revision 16
# baseline (speedup 1.0000x reference)
"""Single-head attention (B=4, S=2048, F=1024) on 8 TRN2 NeuronCores.

All matmuls run as fp32r (11-bit-mantissa fp32; full PE rate, rounding
verified bit-exact against hardware). Probabilities flow through bf16.

Key-split sharding: core c handles batch b=c//2 and KEY half h=c%2
(keys [h*1024, (h+1)*1024)), with ALL 2048 q rows.

Softmax row-invariance folds both input projections of the logits into
ONE matrix applied on the (sharded) key side:
  qp.kp^T = q (Wq^T Wk) k^T + q.(Wq^T bk) + k.(Wk^T bq) + bq.bk
The 2nd and 4th terms are constant per query row -> dropped. So
  logits == q @ k'^T + beta_k,   k' = k @ A^T,  A = Wq^T Wk,
  beta = k @ (Wk^T bq)
A and beta are host-precomputed; the Q projection disappears (raw q^T
is already in lhsT layout) and the K projection halves. beta (std ~1
vs logit std ~32) is added to the logits PSUM on the Vector engine.

Each core emits an UNNORMALIZED partial attention output plus per-row
(max, sum) softmax stats; the host merges the two halves flash-style:
  m = max(m0, m1); out = (o0*e^{m0-m} + o1*e^{m1-m}) /
                         (s0*e^{m0-m} + s1*e^{m1-m}) + q + bv
"""

import numpy as np
from contextlib import ExitStack

import concourse.bass as bass
import concourse.tile as tile
import concourse.mybir as mybir
from concourse import bacc
from concourse.bass_utils import run_bass_kernel_spmd
from concourse.masks import make_identity

B, S, F = 4, 2048, 1024
P = 128
SK = S // 2            # keys per core
FT = F // P            # 8 contraction tiles
GT = F // P            # 8 output-feature tiles
KC = SK // 512         # 2 key chunks of 512
QI = S // P            # 16 q-tiles per core
KB = SK // P           # 8 key blocks
N_CORES = 8

f32 = mybir.dt.float32
f32r = mybir.dt.float32r
bf16 = mybir.dt.bfloat16
AX = mybir.AxisListType.X
AF = mybir.ActivationFunctionType

_CACHE = {}
WARM = 30


def _build(repeat=1):
    nc = bacc.Bacc("TRN2", target_bir_lowering=False, debug=False,
                   num_devices=N_CORES)
    qT = nc.dram_tensor("qT", [F, S], f32, kind="ExternalInput").ap()
    kT = nc.dram_tensor("kT", [F, SK], f32, kind="ExternalInput").ap()
    vT = nc.dram_tensor("vT", [F, SK], f32, kind="ExternalInput").ap()
    mT = nc.dram_tensor("mT", [F, F], f32, kind="ExternalInput").ap()
    wvT = nc.dram_tensor("wvT", [F, F], f32, kind="ExternalInput").ap()
    betab = nc.dram_tensor("betab", [P, SK], f32, kind="ExternalInput").ap()
    out = nc.dram_tensor("out", [S, F], f32, kind="ExternalOutput").ap()
    ms = nc.dram_tensor("ms", [S, 2], f32, kind="ExternalOutput").ap()

    with tile.TileContext(nc) as tc, ExitStack() as ctx:
      consts = ctx.enter_context(tc.tile_pool(name="consts", bufs=1))
      wpool = ctx.enter_context(tc.tile_pool(name="w", bufs=12))
      xin = ctx.enter_context(tc.tile_pool(name="xin", bufs=16))
      proj = ctx.enter_context(tc.tile_pool(name="proj", bufs=1))
      sm = ctx.enter_context(tc.tile_pool(name="sm", bufs=2))
      stats = ctx.enter_context(tc.tile_pool(name="stats", bufs=2))
      outp = ctx.enter_context(tc.tile_pool(name="outp", bufs=2))
      psA = ctx.enter_context(tc.tile_pool(name="psA", bufs=4, space="PSUM"))
      psT = ctx.enter_context(tc.tile_pool(name="psT", bufs=2, space="PSUM"))
      psV = ctx.enter_context(tc.tile_pool(name="psV", bufs=2, space="PSUM"))
      for _rep in range(repeat):
        ident = consts.tile([P, P], bf16, tag="ident")
        make_identity(nc, ident)
        beta_sb = consts.tile([P, SK], f32, tag="beta")

        kpT = [proj.tile([P, SK], f32r, tag=f"kpT{g}", name=f"kpT{g}")
               for g in range(GT)]
        vp = [proj.tile([P, F], bf16, tag=f"vp{i}", name=f"vp{i}")
              for i in range(KB)]

        # raw q^T streams through a 3-chunk ring (chunk = 2 q-tiles x 8 gt)
        qs = {}

        def load_qchunk(qc):
            t = [proj.tile([P, 256], f32r, tag="qs", bufs=16, name="qs")
                 for _ in range(GT)]
            for gt in range(GT):
                nc.sync.dma_start(
                    t[gt][:],
                    qT[gt * P:(gt + 1) * P, qc * 256:(qc + 1) * 256].bitcast(f32r))
            qs[qc] = t

        # ---- k' projection: kpT[f, t] = (A @ kT)[f, t] over the key half ----
        # DMA bandwidth is the startup bottleneck (all queues share ~350GB/s),
        # so loads are issued in first-needed order at sub-tile granularity:
        # mT cols [gt*128,gt*128+128) feed psum group gt of each chunk.
        m_sb = [wpool.tile([P, F], f32r, tag="w", name="w") for _ in range(FT)]
        kxs = [[xin.tile([P, 512], f32r, tag="xin", name="xin")
                for _ in range(FT)] for _ in range(KC)]
        for ft in range(FT):
            nc.sync.dma_start(
                m_sb[ft][:, 0:256], mT[ft * P:(ft + 1) * P, 0:256].bitcast(f32r))
            nc.sync.dma_start(
                kxs[0][ft][:], kT[ft * P:(ft + 1) * P, 0:512].bitcast(f32r))
        for cb in range(1, 4):
            for ft in range(FT):
                nc.sync.dma_start(
                    m_sb[ft][:, cb * 256:(cb + 1) * 256],
                    mT[ft * P:(ft + 1) * P, cb * 256:(cb + 1) * 256].bitcast(f32r))
        for ft in range(FT):
            nc.sync.dma_start(
                kxs[1][ft][:], kT[ft * P:(ft + 1) * P, 512:1024].bitcast(f32r))
        # keep the PE clock ramped through the DMA-paced startup
        for _ in range(WARM):
            wps = psV.tile([P, P], bf16, tag="pvps")
            nc.tensor.matmul(wps[:], ident[:], ident[:], is_transpose=True)
        vxs = None
        wv_sb = None
        for sc in range(KC):
            for gt in range(GT):
                ps = psA.tile([P, 512], f32, tag="mmps")
                for ft in range(FT):
                    nc.tensor.matmul(ps[:], m_sb[ft][:, gt * P:(gt + 1) * P],
                                     kxs[sc][ft][:], start=(ft == 0),
                                     stop=(ft == FT - 1))
                nc.scalar.copy(kpT[gt][:, sc * 512:(sc + 1) * 512], ps[:])
            if sc == 0:
                # later-phase loads issue after the first k-chunk group so
                # they overlap k'-projection compute without delaying it.
                # wv/vx interleaved (vproj chunk 0 needs both), then vx1,
                # then the first three q chunks.
                vxs = [[xin.tile([P, 512], f32r, tag="xin", name="xin")
                        for _ in range(FT)] for _ in range(KC)]
                wv_sb = [wpool.tile([P, F], f32r, tag="wv", name="wv")
                         for _ in range(FT)]
                for ft in range(FT):
                    nc.sync.dma_start(
                        wv_sb[ft][:, 0:512],
                        wvT[ft * P:(ft + 1) * P, 0:512].bitcast(f32r))
                    nc.sync.dma_start(
                        vxs[0][ft][:, 0:256],
                        vT[ft * P:(ft + 1) * P, 0:256].bitcast(f32r))
                for ft in range(FT):
                    nc.sync.dma_start(
                        wv_sb[ft][:, 512:1024],
                        wvT[ft * P:(ft + 1) * P, 512:1024].bitcast(f32r))
                    nc.sync.dma_start(
                        vxs[0][ft][:, 256:512],
                        vT[ft * P:(ft + 1) * P, 256:512].bitcast(f32r))
                for vh in range(2):
                    for ft in range(FT):
                        nc.sync.dma_start(
                            vxs[1][ft][:, vh * 256:(vh + 1) * 256],
                            vT[ft * P:(ft + 1) * P,
                               512 + vh * 256:512 + (vh + 1) * 256].bitcast(f32r))
                nc.sync.dma_start(beta_sb[:], betab)
                for qc in range(2):
                    load_qchunk(qc)

        # ---- V projection over this core's key half (bf16 out) ----
        for sc in range(KC):
            vx = vxs[sc]
            for blk in range(4):
                kb = sc * 4 + blk
                for gc in range(2):
                    ps = psA.tile([P, 512], f32, tag="mmps")
                    for ft in range(FT):
                        nc.tensor.matmul(
                            ps[:], vx[ft][:, blk * P:(blk + 1) * P],
                            wv_sb[ft][:, gc * 512:(gc + 1) * 512],
                            start=(ft == 0), stop=(ft == FT - 1))
                    nc.vector.tensor_copy(vp[kb][:, gc * 512:(gc + 1) * 512],
                                          ps[:])

        # ---- attention over the local key half, pipelined over q-tiles ----
        def emit_logits(qi):
            qt = qs[qi // 2]
            qo = (qi % 2) * P
            lps = []
            m4 = stats.tile([P, KC], f32, tag="m4")
            for kc in range(KC):
                ps = psA.tile([P, 512], f32, tag="mmps")
                for gt in range(GT):
                    nc.tensor.matmul(ps[:], qt[gt][:, qo:qo + P],
                                     kpT[gt][:, kc * 512:(kc + 1) * 512],
                                     start=(gt == 0), stop=(gt == GT - 1))
                nc.vector.tensor_add(ps[:], ps[:],
                                     beta_sb[:, kc * 512:(kc + 1) * 512])
                nc.vector.reduce_max(m4[:, kc:kc + 1], ps[:], axis=AX)
                lps.append(ps)
            return lps, m4

        cur = emit_logits(0)
        for qi in range(QI):
            lps, m4 = cur
            if qi % 2 == 0 and qi // 2 + 2 < S // 256:
                load_qchunk(qi // 2 + 2)
            negm = stats.tile([P, 1], f32, tag="negm")
            nc.vector.reduce_max(negm[:], m4[:], axis=AX, negate=True)
            esc = sm.tile([P, SK], bf16, tag="esc", bufs=1)
            ssum2 = stats.tile([P, KC], f32, tag="ssum2")
            for kc in range(KC):
                nc.scalar.activation(esc[:, kc * 512:(kc + 1) * 512], lps[kc][:],
                                     AF.Exp, bias=negm[:, 0:1], scale=1.0,
                                     accum_out=ssum2[:, kc:kc + 1])
            if qi + 1 < QI:
                cur = emit_logits(qi + 1)
            ssum = stats.tile([P, 1], f32, tag="ssum")
            nc.vector.reduce_sum(ssum[:], ssum2[:], axis=AX)
            msb = stats.tile([P, 2], f32, tag="msb")
            nc.vector.tensor_copy(msb[:, 0:1], negm[:])
            nc.vector.tensor_copy(msb[:, 1:2], ssum[:])
            nc.sync.dma_start(ms[qi * P:(qi + 1) * P, :], msb[:])

            escT = sm.tile([P, SK], bf16, tag="escT")
            for t4 in range(KC):
                tp = psT.tile([P, 512], bf16, tag="tpps")
                for j in range(4):
                    nc.tensor.matmul(tp[:, j * P:(j + 1) * P],
                                     esc[:, (t4 * 4 + j) * P:(t4 * 4 + j + 1) * P],
                                     ident[:], is_transpose=True,
                                     start=(j == 0), stop=(j == 3))
                nc.scalar.copy(escT[:, t4 * 512:(t4 + 1) * 512], tp[:])

            for gc in range(2):
                pvps = psV.tile([P, 512], f32, tag="pvps")
                for kb in range(KB):
                    nc.tensor.matmul(pvps[:], escT[:, kb * P:(kb + 1) * P],
                                     vp[kb][:, gc * 512:(gc + 1) * 512],
                                     start=(kb == 0), stop=(kb == KB - 1))
                ob = outp.tile([P, 512], f32, tag="ob")
                nc.scalar.copy(ob[:], pvps[:])
                nc.sync.dma_start(
                    out[qi * P:(qi + 1) * P, gc * 512:(gc + 1) * 512], ob[:])

    nc.compile()
    return nc


def _round_f32r(x):
    xi = np.ascontiguousarray(x, dtype=np.float32).view(np.uint32)
    r = (xi + np.uint32(0x800)) & np.uint32(0xFFFFF000)
    return r.view(np.float32)


def _get_nc(repeat=1):
    key = f"nc{repeat}"
    if key not in _CACHE:
        _CACHE[key] = _build(repeat)
    return _CACHE[key]


def _make_in_maps(q, k, v, Wq, bq, Wk, bk, Wv, bv):
    q = np.ascontiguousarray(q, np.float32)
    k = np.ascontiguousarray(k, np.float32)
    v = np.ascontiguousarray(v, np.float32)
    Wq64 = np.asarray(Wq, np.float64)
    Wk64 = np.asarray(Wk, np.float64)
    bq64 = np.asarray(bq, np.float64)
    mTh = _round_f32r((Wk64.T @ Wq64).astype(np.float32))
    u = Wk64.T @ bq64
    wvT = _round_f32r(np.ascontiguousarray(Wv.T))
    qT = [_round_f32r(np.ascontiguousarray(q[b].T)) for b in range(B)]
    in_maps = []
    for c in range(N_CORES):
        b, h = divmod(c, 2)
        kh = k[b, h * SK:(h + 1) * SK, :]
        kT_c = _round_f32r(np.ascontiguousarray(kh.T))
        vT_c = _round_f32r(np.ascontiguousarray(v[b, h * SK:(h + 1) * SK, :].T))
        beta = (kh.astype(np.float64) @ u).astype(np.float32)
        betab = np.ascontiguousarray(np.broadcast_to(beta[None, :], (P, SK)))
        in_maps.append({
            "qT": qT[b], "kT": kT_c, "vT": vT_c,
            "mT": mTh, "wvT": wvT, "betab": betab,
        })
    return in_maps


def _execute(in_maps, trace=False):
    nc = _get_nc()
    return run_bass_kernel_spmd(nc, in_maps, list(range(N_CORES)), trace=trace)


def _merge(results, q, bv):
    """Flash-style merge of the two key-half partials per batch."""
    out = np.empty((B, S, F), np.float32)
    bv64 = np.asarray(bv, np.float64)
    for b in range(B):
        r0, r1 = results[2 * b], results[2 * b + 1]
        o0 = r0["out"].astype(np.float64)
        o1 = r1["out"].astype(np.float64)
        m0 = -r0["ms"][:, 0].astype(np.float64)
        m1 = -r1["ms"][:, 0].astype(np.float64)
        s0 = r0["ms"][:, 1].astype(np.float64)
        s1 = r1["ms"][:, 1].astype(np.float64)
        m = np.maximum(m0, m1)
        a0 = np.exp(m0 - m)
        a1 = np.exp(m1 - m)
        num = o0 * a0[:, None] + o1 * a1[:, None]
        den = s0 * a0 + s1 * a1
        out[b] = (num / den[:, None] + q[b].astype(np.float64) + bv64
                  ).astype(np.float32)
    return out


def kernel(q, k, v, Wq, bq, Wk, bk, Wv, bv):
    q = np.ascontiguousarray(q, np.float32)
    in_maps = _make_in_maps(q, k, v, Wq, bq, Wk, bk, Wv, bv)
    res = _execute(in_maps)
    return _merge(res.results, q, bv)
